# revision 11
# baseline (speedup 1.0000x reference)
"""Trainium2 Bass kernel for nn_Attention (dense transformer attention).

Math (per batch n, head h):
  q' = q_h @ Wq.T ; k' = k_h @ Wk.T ; v' = v_h @ Wv.T
  S = (q' k'^T)/32 ; P = softmax_k(S) ; out_h = P v'
  final = concat_h(out_h) @ Wout.T + bout

Device-side reformulation (associativity, exact in real arithmetic):
  S   = Q @ Wc @ K^T      with Wc = (Wq.T @ Wk)/32   (K unprojected!)
  U^T = [V | 1]^T @ exp(S)^T   -> rows 0..63 = V^T exp(S)^T, row 64 = denoms
  out_h^T = (Wv @ U^T[0:64]) / denom    (Wv projection after attention)
  final^T = Wout @ attn^T + bout

Numerics: everything that feeds the PE is bf16 (|S| <= ~2.5 so exp is tame;
measured end-to-end absmax rel err ~4e-3, tolerance 2e-2). PSUM accumulation
stays f32, final output written f32.

Sharding: sequence-parallel over the 2048 queries -> 8 cores x 256 queries.

Schedule: the 16 (batch, head-pair) slots are software-pipelined. In slot p
the PE computes S^T(p) (feeding ACT's exp, the bottleneck engine) and then
the U^T/Wv/normalize flush of slot p-1, whose exp outputs finished during
slot p-1. fc_out for batch n runs in slot (n+1, 0) after that slot's S^T,
so ACT keeps a full pair of exp work during fc_out's matmuls.

Host-side packing makes every DMA big-descriptor and cuts the input-tensor
count to 4:
  kT    (N, E, L)                bf16 keys^T        4 KiB descriptors
  qT    (N, E, LQ)               bf16 query^T slice 512 B descriptors
  vpack (N, 128, NCHUNK, H, 65)  bf16 [V | 1] token-partition-major; one
                                 8 KiB contiguous descriptor per partition
                                 per 4-chunk load
  wpack (128, 8584)              bf16 all weights in one tensor:
        [:, 0:128]    blockdiag(Wc, Wc)
        [0:64, 128:384]  [Wv.T | 0] and [0 | Wv.T]  (head0/head1 PE placement)
        [:, 384:392]  bias (bout partition-major)
        [:, 392:8584] Wout^T as [128, ec, o]
"""

import sys

for p in ("/opt/trn_rl_repo",):
    if p not in sys.path:
        sys.path.insert(0, p)

import numpy as np

import os as _os

N = 2
L = 2048
E = 1024
H = 16
D = 64
NCORES = int(_os.environ.get("BASS_KERNEL_NCORES", "8"))
LQ = L // NCORES          # queries per core
LQB = 256                 # query-tile size (PSUM-sized)
QT = LQ // LQB            # query tiles per core
NPAIR = H // 2            # 8 head-pairs per batch
NCHUNK = L // 128         # 16 key chunks of 128 tokens
WCOL_WV = 128
WCOL_BIAS = 384
WCOL_WOUT = 392
WCOLS = 392 + E * (E // 128)

REPEAT = int(_os.environ.get("BASS_KERNEL_REPEAT", "1"))


def build_nc():
    import concourse.bass as bass
    import concourse.bacc as bacc
    import concourse.mybir as mybir
    import concourse.tile as tile

    f32 = mybir.dt.float32
    bf16 = mybir.dt.bfloat16
    EXP = mybir.ActivationFunctionType.Exp
    MUL = mybir.AluOpType.mult
    ADD = mybir.AluOpType.add

    nc = bacc.Bacc(None, target_bir_lowering=False, enable_partition_id=False)

    kT = nc.dram_tensor("kT", [N, E, L], bf16, kind="ExternalInput")
    qT = nc.dram_tensor("qT", [N, E, LQ], bf16, kind="ExternalInput")
    vpack = nc.dram_tensor("vpack", [N, 128, NCHUNK, H, D + 1], bf16,
                           kind="ExternalInput")
    wpack = nc.dram_tensor("wpack", [128, WCOLS], bf16, kind="ExternalInput")
    outT = nc.dram_tensor("outT", [N, E, LQ], f32, kind="ExternalOutput")

    with tile.TileContext(nc) as tc:
        with (
            tc.tile_pool(name="const", bufs=1) as const,
            tc.tile_pool(name="vio", bufs=2) as vio,
            tc.tile_pool(name="io", bufs=2) as io,
            tc.tile_pool(name="work", bufs=3) as work,
            tc.tile_pool(name="expp", bufs=4) as expp,
            tc.tile_pool(name="attnp", bufs=QT + 1) as attnp,
            tc.tile_pool(name="psT", bufs=2, space="PSUM") as psT,
            tc.tile_pool(name="pu", bufs=2, space="PSUM") as pu,
            tc.tile_pool(name="psmall", bufs=2, space="PSUM") as psmall,
        ):
            # --- persistent constants: one DMA for all weights ---
            wpack_sb = const.tile([128, WCOLS], bf16)
            nc.sync.dma_start(wpack_sb, wpack[:, :])
            wqk2_sb = wpack_sb[:, 0:128]

            bias_sb = const.tile([128, E // 128], f32)
            nc.vector.tensor_copy(bias_sb, wpack_sb[:, WCOL_BIAS:WCOL_WOUT])
            ones_sb = const.tile([128, 128], bf16)
            nc.vector.memset(ones_sb, 1.0)

            import contextlib

            def load_k(n, h2):
                kT2 = io.tile([128, L], bf16, tag="kT2")
                nc.sync.dma_start(kT2, kT[n, 128 * h2 : 128 * (h2 + 1), :])
                return kT2

            def load_q(n, h2, qt):
                qT2 = io.tile([128, LQB], bf16, tag="qT2")
                nc.sync.dma_start(
                    qT2,
                    qT[n, 128 * h2 : 128 * (h2 + 1), LQB * qt : LQB * (qt + 1)],
                )
                return qT2

            def load_v(n):
                # 4 DMAs of 4 chunks each so early U^T chunks never wait on
                # the full 8 MiB load
                v_sb = vio.tile([128, NCHUNK, H, D + 1], bf16, tag="v",
                                name=f"v_sb_{n}")
                for g in range(4):
                    nc.sync.dma_start(
                        v_sb[:, 4 * g : 4 * (g + 1)], vpack[n, :, 4 * g : 4 * (g + 1)]
                    )
                return v_sb

            def score_phase(kT2, qT2):
                """Q'' projection, S^T matmuls and exp for one head pair."""
                pq = psmall.tile([128, LQB], f32, tag="small")
                nc.tensor.matmul(pq, wqk2_sb, qT2, start=True, stop=True)
                q2sb = work.tile([128, LQB], bf16, tag="q2sb")
                with nc.allow_low_precision("bf16 attention pipeline"):
                    nc.vector.tensor_copy(q2sb, pq)

                expS0 = expp.tile([128, NCHUNK, LQB], bf16, tag="expS")
                expS1 = expp.tile([128, NCHUNK, LQB], bf16, tag="expS")
                exps = (expS0, expS1)
                for rr in range(4):
                    sTs = []
                    for hh in range(2):
                        hs = slice(64 * hh, 64 * hh + 64)
                        sT = psT.tile([128, 4, LQB], f32, tag="sT")
                        sTs.append(sT)
                        for c in range(4):
                            ch = rr * 4 + c
                            nc.tensor.matmul(
                                sT[:, c, :],
                                kT2[hs, 128 * ch : 128 * (ch + 1)],
                                q2sb[hs, :],
                                start=True, stop=True,
                            )
                    for hh in range(2):
                        with nc.allow_low_precision("bf16 exp(S)"):
                            nc.scalar.activation(
                                exps[hh][:, rr * 4 : rr * 4 + 4, :],
                                sTs[hh][:, :, :], EXP,
                            )
                return exps

            def flush_pair(n, h2, v_sb, exps, attn_sb):
                """U^T accumulate, Wv projection, softmax normalize into
                attn_sb for a pair whose exp outputs are ready."""
                r2_sb = work.tile([65, 2, LQB], bf16, tag="r2")
                u_sbs = []
                for hh in range(2):
                    uT = pu.tile([65, LQB], f32, tag="uT")
                    for ch in range(NCHUNK):
                        nc.tensor.matmul(
                            uT,
                            v_sb[:, ch, 2 * h2 + hh, :],
                            exps[hh][:, ch, :],
                            start=(ch == 0), stop=(ch == NCHUNK - 1),
                        )
                    u_sb = work.tile([65, LQB], bf16, tag="u_sb")
                    u_sbs.append(u_sb)
                    with nc.allow_low_precision("bf16 attention pipeline"):
                        nc.vector.tensor_copy(u_sb, uT)
                        nc.vector.reciprocal(r2_sb[64:65, hh, :], u_sb[64:65, :])
                # Wv projection, head hh placed at partitions 64*hh..64*hh+63
                # via the zero-padded [Wv.T|0]/[0|Wv.T] stationary operands
                up = pu.tile([128, LQB], f32, tag="uT")
                for hh in range(2):
                    nc.tensor.matmul(
                        up,
                        wpack_sb[0:64, WCOL_WV + 128 * hh : WCOL_WV + 128 * (hh + 1)],
                        u_sbs[hh][0:64, :],
                        start=(hh == 0), stop=(hh == 1),
                    )
                # broadcast 1/denom across partitions via PE outer product
                pb = psmall.tile([128, 2, LQB], f32, tag="small")
                nc.tensor.matmul(
                    pb, ones_sb[64:65, :], r2_sb[64:65, :, :],
                    start=True, stop=True,
                )
                b_sb = work.tile([128, 2, LQB], bf16, tag="b_sb")
                with nc.allow_low_precision("bf16 attention pipeline"):
                    nc.vector.tensor_copy(b_sb, pb)
                    nc.vector.tensor_tensor(
                        attn_sb[0:64, h2, :], up[0:64, :], b_sb[0:64, 0, :], MUL,
                    )
                    nc.vector.tensor_tensor(
                        attn_sb[64:128, h2, :], up[64:128, :], b_sb[64:128, 1, :],
                        MUL,
                    )

            def fc_out(n, qt, attn_sb):
                for oc in range(E // 128):
                    po = psmall.tile([128, LQB], f32, tag="small")
                    for ec in range(E // 128):
                        nc.tensor.matmul(
                            po,
                            wpack_sb[:, WCOL_WOUT + E * ec + 128 * oc
                                     : WCOL_WOUT + E * ec + 128 * (oc + 1)],
                            attn_sb[:, ec, :],
                            start=(ec == 0), stop=(ec == E // 128 - 1),
                        )
                    o_sb = work.tile([128, LQB], f32, tag="o_sb")
                    nc.vector.tensor_tensor(
                        o_sb, po,
                        bias_sb[:, oc : oc + 1].to_broadcast((128, LQB)),
                        ADD,
                    )
                    nc.sync.dma_start(
                        outT[n, 128 * oc : 128 * (oc + 1),
                             LQB * qt : LQB * (qt + 1)], o_sb,
                    )

            rep_ctx = (
                tc.For_i(0, REPEAT, 1) if REPEAT > 1 else contextlib.nullcontext()
            )
            with rep_ctx:
                # slot order: batch -> head-pair -> query-tile, so kT2 is
                # loaded once per (n, h2) and reused across query tiles
                slots = [
                    (n, h2, qt)
                    for n in range(N) for h2 in range(NPAIR) for qt in range(QT)
                ]
                v_sbs = {}
                attn_sbs = {}
                v_sbs[0] = load_v(0)
                kT2_cur = load_k(*slots[0][:2])
                kT2_next = None
                loaded_q = load_q(*slots[0])
                prev = None
                for idx, (n, h2, qt) in enumerate(slots):
                    if h2 == 0:
                        attn_sbs[(n, qt)] = attnp.tile(
                            [128, NPAIR, LQB], bf16, tag="attn",
                            name=f"attn_sb_{n}_{qt}",
                        )
                    qT2 = loaded_q
                    kT2_next = kT2_cur
                    if idx + 1 < len(slots):
                        nxt = slots[idx + 1]
                        if nxt[:2] != (n, h2):
                            kT2_next = load_k(*nxt[:2])
                        loaded_q = load_q(*nxt)
                    if n == 0 and h2 == NPAIR - 2 and qt == 0:
                        v_sbs[1] = load_v(1)
                    exps = score_phase(kT2_cur, qT2)
                    if prev is not None:
                        pn, ph2, pqt, pexps = prev
                        flush_pair(
                            pn, ph2, v_sbs[pn], pexps, attn_sbs[(pn, pqt)]
                        )
                        if ph2 == NPAIR - 1:
                            fc_out(pn, pqt, attn_sbs[(pn, pqt)])
                    prev = (n, h2, qt, exps)
                    kT2_cur = kT2_next
                pn, ph2, pqt, pexps = prev
                flush_pair(pn, ph2, v_sbs[pn], pexps, attn_sbs[(pn, pqt)])
                fc_out(pn, pqt, attn_sbs[(pn, pqt)])

    nc.compile()
    return nc


def shard_inputs(values, keys, query, Wv, Wk, Wq, Wout, bout):
    import ml_dtypes

    bf16 = ml_dtypes.bfloat16
    f = np.float32
    values = np.asarray(values, dtype=f)
    keys = np.asarray(keys, dtype=f)
    query = np.asarray(query, dtype=f)
    Wv, Wk, Wq, Wout, bout = (np.asarray(x, dtype=f) for x in (Wv, Wk, Wq, Wout, bout))

    kT_full = np.ascontiguousarray(keys.transpose(0, 2, 1)).astype(bf16)
    qT_full = np.ascontiguousarray(query.transpose(0, 2, 1)).astype(bf16)

    # [V | 1] token-partition-major: vpack[n, p, c, h, :] =
    #   [values[n, c*128+p, h*64:(h+1)*64], 1]
    vpack = np.ones((N, 128, NCHUNK, H, D + 1), dtype=bf16)
    vr = values.reshape(N, NCHUNK, 128, H, D).transpose(0, 2, 1, 3, 4)
    vpack[:, :, :, :, 0:D] = vr.astype(bf16)

    Wc = (Wq.T @ Wk) / np.float32(np.sqrt(E))
    wpack = np.zeros((128, WCOLS), dtype=bf16)
    wpack[0:64, 0:64] = Wc.astype(bf16)
    wpack[64:128, 64:128] = Wc.astype(bf16)
    wvT = Wv.T.astype(bf16)
    wpack[0:64, WCOL_WV : WCOL_WV + 64] = wvT
    wpack[0:64, WCOL_WV + 192 : WCOL_WV + 256] = wvT
    wpack[:, WCOL_BIAS:WCOL_WOUT] = (
        bout.reshape(E // 128, 128).T.astype(bf16)
    )
    # wout block: [p, ec*E + o] = Wout.T[ec*128 + p, o]
    woutT = np.ascontiguousarray(Wout.T).astype(bf16)
    wpack[:, WCOL_WOUT:] = (
        woutT.reshape(E // 128, 128, E).transpose(1, 0, 2).reshape(128, -1)
    )

    in_maps = []
    for c in range(NCORES):
        in_maps.append({
            "kT": kT_full,
            "qT": np.ascontiguousarray(qT_full[:, :, c * LQ : (c + 1) * LQ]),
            "vpack": vpack,
            "wpack": wpack,
        })
    return in_maps


def unshard(results):
    slabs = [np.asarray(r["outT"]).transpose(0, 2, 1) for r in results]
    return np.ascontiguousarray(np.concatenate(slabs, axis=1)).astype(np.float32)


def run_spmd(in_maps, **kwargs):
    from concourse.bass_utils import run_bass_kernel_spmd

    nc = build_nc()
    res = run_bass_kernel_spmd(nc, in_maps, core_ids=list(range(NCORES)), **kwargs)
    return nc, res


def kernel(**inputs):
    in_maps = shard_inputs(
        inputs["values"], inputs["keys"], inputs["query"],
        inputs["Wv"], inputs["Wk"], inputs["Wq"],
        inputs["Wout"], inputs["bout"],
    )
    _, res = run_spmd(in_maps)
    return unshard(res.results)


if __name__ == "__main__":
    rng = np.random.default_rng(0)
    ins = {
        "values": rng.standard_normal((N, L, E), dtype=np.float32),
        "keys": rng.standard_normal((N, L, E), dtype=np.float32),
        "query": rng.standard_normal((N, L, E), dtype=np.float32),
        "Wv": rng.standard_normal((D, D), dtype=np.float32) / 8,
        "Wk": rng.standard_normal((D, D), dtype=np.float32) / 8,
        "Wq": rng.standard_normal((D, D), dtype=np.float32) / 8,
        "Wout": rng.standard_normal((E, E), dtype=np.float32) / 32,
        "bout": rng.standard_normal((E,), dtype=np.float32) * 0.01,
    }
    out = kernel(**ins)
    print("out", out.shape, out.dtype, float(np.abs(out).max()))


# revision 12
# speedup vs baseline: 1.2680x; 1.2680x over previous
"""Trainium2 Bass kernel for nn_Attention (dense transformer attention).

Math (per batch n, head h):
  q' = q_h @ Wq.T ; k' = k_h @ Wk.T ; v' = v_h @ Wv.T
  S = (q' k'^T)/32 ; P = softmax_k(S) ; out_h = P v'
  final = concat_h(out_h) @ Wout.T + bout

Device-side reformulation (associativity, exact in real arithmetic):
  S   = Q @ Wc @ K^T      with Wc = (Wq.T @ Wk)/32   (K unprojected!)
  U^T = [V | 1]^T @ exp(S)^T   -> rows 0..63 = V^T exp(S)^T, row 64 = denoms
  out_h^T = (Wv @ U^T[0:64]) / denom    (Wv projection after attention)
  final^T = Wout @ attn^T + bout

Numerics: everything that feeds the PE is bf16 (|S| <= ~2.5 so exp is tame;
measured end-to-end absmax rel err ~4e-3, tolerance 2e-2). PSUM accumulation
stays f32, final output written f32.

Sharding: sequence-parallel over the 2048 queries -> 8 cores x 256 queries.

Schedule: the 16 (batch, head-pair) slots are software-pipelined. In slot p
the PE computes S^T(p) (feeding ACT's exp, the bottleneck engine) and then
the U^T/Wv/normalize flush of slot p-1, whose exp outputs finished during
slot p-1. fc_out for batch n runs in slot (n+1, 0) after that slot's S^T,
so ACT keeps a full pair of exp work during fc_out's matmuls.

Host-side packing makes every DMA big-descriptor and cuts the input-tensor
count to 4:
  kT    (N, E, L)                bf16 keys^T        4 KiB descriptors
  qT    (N, E, LQ)               bf16 query^T slice 512 B descriptors
  vpack (N, 128, NCHUNK, H, 65)  bf16 [V | 1] token-partition-major; one
                                 8 KiB contiguous descriptor per partition
                                 per 4-chunk load
  wpack (128, 8584)              bf16 all weights in one tensor:
        [:, 0:128]    blockdiag(Wc, Wc)
        [0:64, 128:384]  [Wv.T | 0] and [0 | Wv.T]  (head0/head1 PE placement)
        [:, 384:392]  bias (bout partition-major)
        [:, 392:8584] Wout^T as [128, ec, o]
"""

import sys

for p in ("/opt/trn_rl_repo",):
    if p not in sys.path:
        sys.path.insert(0, p)

import numpy as np

import os as _os

N = 2
L = 2048
E = 1024
H = 16
D = 64
NCORES = int(_os.environ.get("BASS_KERNEL_NCORES", "8"))
LQ = L // NCORES          # queries per core
LQB = 256                 # query-tile size (PSUM-sized)
QT = LQ // LQB            # query tiles per core
NPAIR = H // 2            # 8 head-pairs per batch
NCHUNK = L // 128         # 16 key chunks of 128 tokens
WCOL_WV = 128
WCOL_BIAS = 384
WCOL_WOUT = 392
WCOLS = 392 + E * (E // 128)

# single packed input blob (element offsets, bf16)
OFF_W = 0
OFF_K = OFF_W + 128 * WCOLS
OFF_V = OFF_K + N * E * L
OFF_Q = OFF_V + N * 128 * NCHUNK * H * (D + 1)
BLOB = OFF_Q + N * E * LQ

REPEAT = int(_os.environ.get("BASS_KERNEL_REPEAT", "1"))


def build_nc():
    import concourse.bass as bass
    import concourse.bacc as bacc
    import concourse.mybir as mybir
    import concourse.tile as tile

    f32 = mybir.dt.float32
    bf16 = mybir.dt.bfloat16
    EXP = mybir.ActivationFunctionType.Exp
    MUL = mybir.AluOpType.mult
    ADD = mybir.AluOpType.add

    nc = bacc.Bacc(None, target_bir_lowering=False, enable_partition_id=False)

    blob = nc.dram_tensor("blob", [BLOB], bf16, kind="ExternalInput")
    wpack = blob[OFF_W : OFF_W + 128 * WCOLS].rearrange(
        "(p c) -> p c", p=128, c=WCOLS)
    kT = blob[OFF_K : OFF_K + N * E * L].rearrange(
        "(n e l) -> n e l", n=N, e=E, l=L)
    vpack = blob[OFF_V : OFF_Q].rearrange(
        "(n p c h d) -> n p c h d", n=N, p=128, c=NCHUNK, h=H, d=D + 1)
    qT = blob[OFF_Q : OFF_Q + N * E * LQ].rearrange(
        "(n e l) -> n e l", n=N, e=E, l=LQ)
    outT = nc.dram_tensor("outT", [N, E, LQ], f32, kind="ExternalOutput")

    with tile.TileContext(nc) as tc:
        with (
            tc.tile_pool(name="const", bufs=1) as const,
            tc.tile_pool(name="vio", bufs=2) as vio,
            tc.tile_pool(name="io", bufs=2) as io,
            tc.tile_pool(name="work", bufs=3) as work,
            tc.tile_pool(name="expp", bufs=4) as expp,
            tc.tile_pool(name="attnp", bufs=QT + 1) as attnp,
            tc.tile_pool(name="psT", bufs=2, space="PSUM") as psT,
            tc.tile_pool(name="pu", bufs=2, space="PSUM") as pu,
            tc.tile_pool(name="psmall", bufs=2, space="PSUM") as psmall,
        ):
            # --- persistent constants: one DMA for all weights ---
            wpack_sb = const.tile([128, WCOLS], bf16)
            nc.sync.dma_start(wpack_sb, wpack[:, :])
            wqk2_sb = wpack_sb[:, 0:128]

            bias_sb = const.tile([128, E // 128], f32)
            nc.vector.tensor_copy(bias_sb, wpack_sb[:, WCOL_BIAS:WCOL_WOUT])
            ones_sb = const.tile([128, 128], bf16)
            nc.vector.memset(ones_sb, 1.0)

            import contextlib

            def load_k(n, h2):
                kT2 = io.tile([128, L], bf16, tag="kT2")
                nc.sync.dma_start(kT2, kT[n, 128 * h2 : 128 * (h2 + 1), :])
                return kT2

            def load_q(n, h2, qt):
                qT2 = io.tile([128, LQB], bf16, tag="qT2")
                nc.sync.dma_start(
                    qT2,
                    qT[n, 128 * h2 : 128 * (h2 + 1), LQB * qt : LQB * (qt + 1)],
                )
                return qT2

            def load_v(n):
                # 4 DMAs of 4 chunks each so early U^T chunks never wait on
                # the full 8 MiB load
                v_sb = vio.tile([128, NCHUNK, H, D + 1], bf16, tag="v",
                                name=f"v_sb_{n}")
                for g in range(4):
                    nc.sync.dma_start(
                        v_sb[:, 4 * g : 4 * (g + 1)], vpack[n, :, 4 * g : 4 * (g + 1)]
                    )
                return v_sb

            def score_phase(kT2, qT2):
                """Q'' projection, S^T matmuls and exp for one head pair."""
                pq = psmall.tile([128, LQB], f32, tag="small")
                nc.tensor.matmul(pq, wqk2_sb, qT2, start=True, stop=True)
                q2sb = work.tile([128, LQB], bf16, tag="q2sb")
                with nc.allow_low_precision("bf16 attention pipeline"):
                    nc.vector.tensor_copy(q2sb, pq)

                expS0 = expp.tile([128, NCHUNK, LQB], bf16, tag="expS")
                expS1 = expp.tile([128, NCHUNK, LQB], bf16, tag="expS")
                exps = (expS0, expS1)
                for rr in range(4):
                    sTs = []
                    for hh in range(2):
                        hs = slice(64 * hh, 64 * hh + 64)
                        sT = psT.tile([128, 4, LQB], f32, tag="sT")
                        sTs.append(sT)
                        for c in range(4):
                            ch = rr * 4 + c
                            nc.tensor.matmul(
                                sT[:, c, :],
                                kT2[hs, 128 * ch : 128 * (ch + 1)],
                                q2sb[hs, :],
                                start=True, stop=True,
                            )
                    for hh in range(2):
                        with nc.allow_low_precision("bf16 exp(S)"):
                            nc.scalar.activation(
                                exps[hh][:, rr * 4 : rr * 4 + 4, :],
                                sTs[hh][:, :, :], EXP,
                            )
                return exps

            def flush_pair(n, h2, v_sb, exps, attn_sb):
                """U^T accumulate, Wv projection, softmax normalize into
                attn_sb for a pair whose exp outputs are ready."""
                r2_sb = work.tile([65, 2, LQB], bf16, tag="r2")
                u_sbs = []
                for hh in range(2):
                    uT = pu.tile([65, LQB], f32, tag="uT")
                    for ch in range(NCHUNK):
                        nc.tensor.matmul(
                            uT,
                            v_sb[:, ch, 2 * h2 + hh, :],
                            exps[hh][:, ch, :],
                            start=(ch == 0), stop=(ch == NCHUNK - 1),
                        )
                    u_sb = work.tile([65, LQB], bf16, tag="u_sb")
                    u_sbs.append(u_sb)
                    with nc.allow_low_precision("bf16 attention pipeline"):
                        nc.vector.tensor_copy(u_sb, uT)
                        nc.vector.reciprocal(r2_sb[64:65, hh, :], u_sb[64:65, :])
                # Wv projection, head hh placed at partitions 64*hh..64*hh+63
                # via the zero-padded [Wv.T|0]/[0|Wv.T] stationary operands
                up = pu.tile([128, LQB], f32, tag="uT")
                for hh in range(2):
                    nc.tensor.matmul(
                        up,
                        wpack_sb[0:64, WCOL_WV + 128 * hh : WCOL_WV + 128 * (hh + 1)],
                        u_sbs[hh][0:64, :],
                        start=(hh == 0), stop=(hh == 1),
                    )
                # broadcast 1/denom across partitions via PE outer product
                pb = psmall.tile([128, 2, LQB], f32, tag="small")
                nc.tensor.matmul(
                    pb, ones_sb[64:65, :], r2_sb[64:65, :, :],
                    start=True, stop=True,
                )
                b_sb = work.tile([128, 2, LQB], bf16, tag="b_sb")
                with nc.allow_low_precision("bf16 attention pipeline"):
                    nc.vector.tensor_copy(b_sb, pb)
                    nc.vector.tensor_tensor(
                        attn_sb[0:64, h2, :], up[0:64, :], b_sb[0:64, 0, :], MUL,
                    )
                    nc.vector.tensor_tensor(
                        attn_sb[64:128, h2, :], up[64:128, :], b_sb[64:128, 1, :],
                        MUL,
                    )

            def fc_out(n, qt, attn_sb):
                for oc in range(E // 128):
                    po = psmall.tile([128, LQB], f32, tag="small")
                    for ec in range(E // 128):
                        nc.tensor.matmul(
                            po,
                            wpack_sb[:, WCOL_WOUT + E * ec + 128 * oc
                                     : WCOL_WOUT + E * ec + 128 * (oc + 1)],
                            attn_sb[:, ec, :],
                            start=(ec == 0), stop=(ec == E // 128 - 1),
                        )
                    o_sb = work.tile([128, LQB], f32, tag="o_sb")
                    nc.vector.tensor_tensor(
                        o_sb, po,
                        bias_sb[:, oc : oc + 1].to_broadcast((128, LQB)),
                        ADD,
                    )
                    nc.sync.dma_start(
                        outT[n, 128 * oc : 128 * (oc + 1),
                             LQB * qt : LQB * (qt + 1)], o_sb,
                    )

            rep_ctx = (
                tc.For_i(0, REPEAT, 1) if REPEAT > 1 else contextlib.nullcontext()
            )
            with rep_ctx:
                # slot order: batch -> head-pair -> query-tile, so kT2 is
                # loaded once per (n, h2) and reused across query tiles
                slots = [
                    (n, h2, qt)
                    for n in range(N) for h2 in range(NPAIR) for qt in range(QT)
                ]
                v_sbs = {}
                attn_sbs = {}
                v_sbs[0] = load_v(0)
                kT2_cur = load_k(*slots[0][:2])
                kT2_next = None
                loaded_q = load_q(*slots[0])
                prev = None
                for idx, (n, h2, qt) in enumerate(slots):
                    if h2 == 0:
                        attn_sbs[(n, qt)] = attnp.tile(
                            [128, NPAIR, LQB], bf16, tag="attn",
                            name=f"attn_sb_{n}_{qt}",
                        )
                    qT2 = loaded_q
                    kT2_next = kT2_cur
                    if idx + 1 < len(slots):
                        nxt = slots[idx + 1]
                        if nxt[:2] != (n, h2):
                            kT2_next = load_k(*nxt[:2])
                        loaded_q = load_q(*nxt)
                    if n == 0 and h2 == NPAIR - 2 and qt == 0:
                        v_sbs[1] = load_v(1)
                    exps = score_phase(kT2_cur, qT2)
                    if prev is not None:
                        pn, ph2, pqt, pexps = prev
                        flush_pair(
                            pn, ph2, v_sbs[pn], pexps, attn_sbs[(pn, pqt)]
                        )
                        if ph2 == NPAIR - 1:
                            fc_out(pn, pqt, attn_sbs[(pn, pqt)])
                    prev = (n, h2, qt, exps)
                    kT2_cur = kT2_next
                pn, ph2, pqt, pexps = prev
                flush_pair(pn, ph2, v_sbs[pn], pexps, attn_sbs[(pn, pqt)])
                fc_out(pn, pqt, attn_sbs[(pn, pqt)])

    nc.compile()
    return nc


def shard_inputs(values, keys, query, Wv, Wk, Wq, Wout, bout):
    import ml_dtypes

    bf16 = ml_dtypes.bfloat16
    f = np.float32
    values = np.asarray(values, dtype=f)
    keys = np.asarray(keys, dtype=f)
    query = np.asarray(query, dtype=f)
    Wv, Wk, Wq, Wout, bout = (np.asarray(x, dtype=f) for x in (Wv, Wk, Wq, Wout, bout))

    kT_full = np.ascontiguousarray(keys.transpose(0, 2, 1)).astype(bf16)
    qT_full = np.ascontiguousarray(query.transpose(0, 2, 1)).astype(bf16)

    # [V | 1] token-partition-major: vpack[n, p, c, h, :] =
    #   [values[n, c*128+p, h*64:(h+1)*64], 1]
    vpack = np.ones((N, 128, NCHUNK, H, D + 1), dtype=bf16)
    vr = values.reshape(N, NCHUNK, 128, H, D).transpose(0, 2, 1, 3, 4)
    vpack[:, :, :, :, 0:D] = vr.astype(bf16)

    Wc = (Wq.T @ Wk) / np.float32(np.sqrt(E))
    wpack = np.zeros((128, WCOLS), dtype=bf16)
    wpack[0:64, 0:64] = Wc.astype(bf16)
    wpack[64:128, 64:128] = Wc.astype(bf16)
    wvT = Wv.T.astype(bf16)
    wpack[0:64, WCOL_WV : WCOL_WV + 64] = wvT
    wpack[0:64, WCOL_WV + 192 : WCOL_WV + 256] = wvT
    wpack[:, WCOL_BIAS:WCOL_WOUT] = (
        bout.reshape(E // 128, 128).T.astype(bf16)
    )
    # wout block: [p, ec*E + o] = Wout.T[ec*128 + p, o]
    woutT = np.ascontiguousarray(Wout.T).astype(bf16)
    wpack[:, WCOL_WOUT:] = (
        woutT.reshape(E // 128, 128, E).transpose(1, 0, 2).reshape(128, -1)
    )

    shared = np.concatenate(
        [wpack.ravel(), kT_full.ravel(), vpack.ravel()])
    in_maps = []
    for c in range(NCORES):
        qc = np.ascontiguousarray(qT_full[:, :, c * LQ : (c + 1) * LQ])
        in_maps.append({
            "blob": np.concatenate([shared, qc.ravel()]),
        })
    return in_maps


def unshard(results):
    slabs = [np.asarray(r["outT"]).transpose(0, 2, 1) for r in results]
    return np.ascontiguousarray(np.concatenate(slabs, axis=1)).astype(np.float32)


def run_spmd(in_maps, **kwargs):
    from concourse.bass_utils import run_bass_kernel_spmd

    nc = build_nc()
    res = run_bass_kernel_spmd(nc, in_maps, core_ids=list(range(NCORES)), **kwargs)
    return nc, res


def kernel(**inputs):
    in_maps = shard_inputs(
        inputs["values"], inputs["keys"], inputs["query"],
        inputs["Wv"], inputs["Wk"], inputs["Wq"],
        inputs["Wout"], inputs["bout"],
    )
    _, res = run_spmd(in_maps)
    return unshard(res.results)


if __name__ == "__main__":
    rng = np.random.default_rng(0)
    ins = {
        "values": rng.standard_normal((N, L, E), dtype=np.float32),
        "keys": rng.standard_normal((N, L, E), dtype=np.float32),
        "query": rng.standard_normal((N, L, E), dtype=np.float32),
        "Wv": rng.standard_normal((D, D), dtype=np.float32) / 8,
        "Wk": rng.standard_normal((D, D), dtype=np.float32) / 8,
        "Wq": rng.standard_normal((D, D), dtype=np.float32) / 8,
        "Wout": rng.standard_normal((E, E), dtype=np.float32) / 32,
        "bout": rng.standard_normal((E,), dtype=np.float32) * 0.01,
    }
    out = kernel(**ins)
    print("out", out.shape, out.dtype, float(np.abs(out).max()))


# revision 14
# speedup vs baseline: 4.6804x; 3.6910x over previous
"""Trainium2 Bass kernel for nn_Attention (dense transformer attention).

Math (per batch n, head h):
  q' = q_h @ Wq.T ; k' = k_h @ Wk.T ; v' = v_h @ Wv.T
  S = (q' k'^T)/32 ; P = softmax_k(S) ; out_h = P v'
  final = concat_h(out_h) @ Wout.T + bout

Device-side reformulation (associativity, exact in real arithmetic):
  S   = Q @ Wc @ K^T      with Wc = (Wq.T @ Wk)/32   (K unprojected!)
  U^T = [V | 1]^T @ exp(S)^T   -> rows 0..63 = V^T exp(S)^T, row 64 = denoms
  out_h^T = (Wv @ U^T[0:64]) / denom    (Wv projection after attention)
  final^T = Wout @ attn^T + bout

Numerics: everything that feeds the PE is bf16 (|S| <= ~2.5 so exp is tame;
measured end-to-end absmax rel err ~4e-3, tolerance 2e-2). PSUM accumulation
stays f32, final output written f32.

Sharding: sequence-parallel over the 2048 queries -> 8 cores x 256 queries.

Schedule: the 16 (batch, head-pair) slots are software-pipelined. In slot p
the PE computes S^T(p) (feeding ACT's exp, the bottleneck engine) and then
the U^T/Wv/normalize flush of slot p-1, whose exp outputs finished during
slot p-1. fc_out for batch n runs in slot (n+1, 0) after that slot's S^T,
so ACT keeps a full pair of exp work during fc_out's matmuls.

Host-side packing makes every DMA big-descriptor and cuts the input-tensor
count to 4:
  kT    (N, E, L)                bf16 keys^T        4 KiB descriptors
  qT    (N, E, LQ)               bf16 query^T slice 512 B descriptors
  vpack (N, 128, NCHUNK, H, 65)  bf16 [V | 1] token-partition-major; one
                                 8 KiB contiguous descriptor per partition
                                 per 4-chunk load
  wpack (128, 8584)              bf16 all weights in one tensor:
        [:, 0:128]    blockdiag(Wc, Wc)
        [0:64, 128:384]  [Wv.T | 0] and [0 | Wv.T]  (head0/head1 PE placement)
        [:, 384:392]  bias (bout partition-major)
        [:, 392:8584] Wout^T as [128, ec, o]
"""

import sys

for p in ("/opt/trn_rl_repo",):
    if p not in sys.path:
        sys.path.insert(0, p)

import numpy as np

import os as _os

N = 2
L = 2048
E = 1024
H = 16
D = 64
NCORES = int(_os.environ.get("BASS_KERNEL_NCORES", "4"))
LQ = L // NCORES          # queries per core
LQB = 256                 # query-tile size (PSUM-sized)
QT = LQ // LQB            # query tiles per core
NPAIR = H // 2            # 8 head-pairs per batch
NCHUNK = L // 128         # 16 key chunks of 128 tokens
WCOL_WV = 128
WCOL_BIAS = 384
WCOL_WOUT = 392
WCOLS = 392 + E * (E // 128)

# single packed input blob (element offsets, bf16)
OFF_W = 0
OFF_K = OFF_W + 128 * WCOLS
OFF_V = OFF_K + N * E * L
OFF_Q = OFF_V + N * 128 * NCHUNK * H * (D + 1)
BLOB = OFF_Q + N * E * LQ

REPEAT = int(_os.environ.get("BASS_KERNEL_REPEAT", "1"))


def build_nc():
    import concourse.bass as bass
    import concourse.bacc as bacc
    import concourse.mybir as mybir
    import concourse.tile as tile

    f32 = mybir.dt.float32
    bf16 = mybir.dt.bfloat16
    EXP = mybir.ActivationFunctionType.Exp
    MUL = mybir.AluOpType.mult
    ADD = mybir.AluOpType.add

    nc = bacc.Bacc(None, target_bir_lowering=False, enable_partition_id=False)

    blob = nc.dram_tensor("blob", [BLOB], bf16, kind="ExternalInput")
    wpack = blob[OFF_W : OFF_W + 128 * WCOLS].rearrange(
        "(p c) -> p c", p=128, c=WCOLS)
    kT = blob[OFF_K : OFF_K + N * E * L].rearrange(
        "(n e l) -> n e l", n=N, e=E, l=L)
    vpack = blob[OFF_V : OFF_Q].rearrange(
        "(n p c h d) -> n p c h d", n=N, p=128, c=NCHUNK, h=H, d=D + 1)
    qT = blob[OFF_Q : OFF_Q + N * E * LQ].rearrange(
        "(n e l) -> n e l", n=N, e=E, l=LQ)
    outT = nc.dram_tensor("outT", [N, E, LQ], f32, kind="ExternalOutput")

    with tile.TileContext(nc) as tc:
        with (
            tc.tile_pool(name="const", bufs=1) as const,
            tc.tile_pool(name="vio", bufs=2) as vio,
            tc.tile_pool(name="io", bufs=2) as io,
            tc.tile_pool(name="work", bufs=3) as work,
            tc.tile_pool(name="expp", bufs=4) as expp,
            tc.tile_pool(name="attnp", bufs=QT + 1) as attnp,
            tc.tile_pool(name="psT", bufs=2, space="PSUM") as psT,
            tc.tile_pool(name="pu", bufs=2, space="PSUM") as pu,
            tc.tile_pool(name="psmall", bufs=2, space="PSUM") as psmall,
        ):
            # --- persistent constants: one DMA for all weights ---
            wpack_sb = const.tile([128, WCOLS], bf16)
            nc.sync.dma_start(wpack_sb, wpack[:, :])
            wqk2_sb = wpack_sb[:, 0:128]

            bias_sb = const.tile([128, E // 128], f32)
            nc.vector.tensor_copy(bias_sb, wpack_sb[:, WCOL_BIAS:WCOL_WOUT])
            ones_sb = const.tile([128, 128], bf16)
            nc.vector.memset(ones_sb, 1.0)

            import contextlib

            def load_k(n, h2):
                kT2 = io.tile([128, L], bf16, tag="kT2")
                nc.sync.dma_start(kT2, kT[n, 128 * h2 : 128 * (h2 + 1), :])
                return kT2

            def load_q(n, h2, qt):
                qT2 = io.tile([128, LQB], bf16, tag="qT2")
                nc.sync.dma_start(
                    qT2,
                    qT[n, 128 * h2 : 128 * (h2 + 1), LQB * qt : LQB * (qt + 1)],
                )
                return qT2

            def load_v(n):
                # 4 DMAs of 4 chunks each so early U^T chunks never wait on
                # the full 8 MiB load
                v_sb = vio.tile([128, NCHUNK, H, D + 1], bf16, tag="v",
                                name=f"v_sb_{n}")
                for g in range(4):
                    nc.sync.dma_start(
                        v_sb[:, 4 * g : 4 * (g + 1)], vpack[n, :, 4 * g : 4 * (g + 1)]
                    )
                return v_sb

            def score_phase(kT2, qT2):
                """Q'' projection, S^T matmuls and exp for one head pair."""
                pq = psmall.tile([128, LQB], f32, tag="small")
                nc.tensor.matmul(pq, wqk2_sb, qT2, start=True, stop=True)
                q2sb = work.tile([128, LQB], bf16, tag="q2sb")
                with nc.allow_low_precision("bf16 attention pipeline"):
                    nc.vector.tensor_copy(q2sb, pq)

                expS0 = expp.tile([128, NCHUNK, LQB], bf16, tag="expS")
                expS1 = expp.tile([128, NCHUNK, LQB], bf16, tag="expS")
                exps = (expS0, expS1)
                for rr in range(4):
                    sTs = []
                    for hh in range(2):
                        hs = slice(64 * hh, 64 * hh + 64)
                        sT = psT.tile([128, 4, LQB], f32, tag="sT")
                        sTs.append(sT)
                        for c in range(4):
                            ch = rr * 4 + c
                            nc.tensor.matmul(
                                sT[:, c, :],
                                kT2[hs, 128 * ch : 128 * (ch + 1)],
                                q2sb[hs, :],
                                start=True, stop=True,
                            )
                    for hh in range(2):
                        with nc.allow_low_precision("bf16 exp(S)"):
                            nc.scalar.activation(
                                exps[hh][:, rr * 4 : rr * 4 + 4, :],
                                sTs[hh][:, :, :], EXP,
                            )
                return exps

            def flush_pair(n, h2, v_sb, exps, attn_sb):
                """U^T accumulate, Wv projection, softmax normalize into
                attn_sb for a pair whose exp outputs are ready."""
                r2_sb = work.tile([65, 2, LQB], bf16, tag="r2")
                u_sbs = []
                for hh in range(2):
                    uT = pu.tile([65, LQB], f32, tag="uT")
                    for ch in range(NCHUNK):
                        nc.tensor.matmul(
                            uT,
                            v_sb[:, ch, 2 * h2 + hh, :],
                            exps[hh][:, ch, :],
                            start=(ch == 0), stop=(ch == NCHUNK - 1),
                        )
                    u_sb = work.tile([65, LQB], bf16, tag="u_sb")
                    u_sbs.append(u_sb)
                    with nc.allow_low_precision("bf16 attention pipeline"):
                        nc.vector.tensor_copy(u_sb, uT)
                        nc.vector.reciprocal(r2_sb[64:65, hh, :], u_sb[64:65, :])
                # Wv projection, head hh placed at partitions 64*hh..64*hh+63
                # via the zero-padded [Wv.T|0]/[0|Wv.T] stationary operands
                up = pu.tile([128, LQB], f32, tag="uT")
                for hh in range(2):
                    nc.tensor.matmul(
                        up,
                        wpack_sb[0:64, WCOL_WV + 128 * hh : WCOL_WV + 128 * (hh + 1)],
                        u_sbs[hh][0:64, :],
                        start=(hh == 0), stop=(hh == 1),
                    )
                # broadcast 1/denom across partitions via PE outer product
                pb = psmall.tile([128, 2, LQB], f32, tag="small")
                nc.tensor.matmul(
                    pb, ones_sb[64:65, :], r2_sb[64:65, :, :],
                    start=True, stop=True,
                )
                b_sb = work.tile([128, 2, LQB], bf16, tag="b_sb")
                with nc.allow_low_precision("bf16 attention pipeline"):
                    nc.vector.tensor_copy(b_sb, pb)
                    nc.vector.tensor_tensor(
                        attn_sb[0:64, h2, :], up[0:64, :], b_sb[0:64, 0, :], MUL,
                    )
                    nc.vector.tensor_tensor(
                        attn_sb[64:128, h2, :], up[64:128, :], b_sb[64:128, 1, :],
                        MUL,
                    )

            def fc_out(n, qt, attn_sb):
                for oc in range(E // 128):
                    po = psmall.tile([128, LQB], f32, tag="small")
                    for ec in range(E // 128):
                        nc.tensor.matmul(
                            po,
                            wpack_sb[:, WCOL_WOUT + E * ec + 128 * oc
                                     : WCOL_WOUT + E * ec + 128 * (oc + 1)],
                            attn_sb[:, ec, :],
                            start=(ec == 0), stop=(ec == E // 128 - 1),
                        )
                    o_sb = work.tile([128, LQB], f32, tag="o_sb")
                    nc.vector.tensor_tensor(
                        o_sb, po,
                        bias_sb[:, oc : oc + 1].to_broadcast((128, LQB)),
                        ADD,
                    )
                    nc.sync.dma_start(
                        outT[n, 128 * oc : 128 * (oc + 1),
                             LQB * qt : LQB * (qt + 1)], o_sb,
                    )

            rep_ctx = (
                tc.For_i(0, REPEAT, 1) if REPEAT > 1 else contextlib.nullcontext()
            )
            with rep_ctx:
                # slot order: batch -> head-pair -> query-tile, so kT2 is
                # loaded once per (n, h2) and reused across query tiles
                slots = [
                    (n, h2, qt)
                    for n in range(N) for h2 in range(NPAIR) for qt in range(QT)
                ]
                v_sbs = {}
                attn_sbs = {}
                # first pair's (small) loads go ahead of the 8 MiB v load so
                # the S^T pipeline starts immediately
                kT2_cur = load_k(*slots[0][:2])
                kT2_next = None
                loaded_q = load_q(*slots[0])
                v_sbs[0] = load_v(0)
                prev = None
                for idx, (n, h2, qt) in enumerate(slots):
                    if h2 == 0:
                        attn_sbs[(n, qt)] = attnp.tile(
                            [128, NPAIR, LQB], bf16, tag="attn",
                            name=f"attn_sb_{n}_{qt}",
                        )
                    qT2 = loaded_q
                    kT2_next = kT2_cur
                    if idx + 1 < len(slots):
                        nxt = slots[idx + 1]
                        if nxt[:2] != (n, h2):
                            kT2_next = load_k(*nxt[:2])
                        loaded_q = load_q(*nxt)
                    if n == 0 and h2 == NPAIR - 2 and qt == 0:
                        v_sbs[1] = load_v(1)
                    exps = score_phase(kT2_cur, qT2)
                    if prev is not None:
                        pn, ph2, pqt, pexps = prev
                        flush_pair(
                            pn, ph2, v_sbs[pn], pexps, attn_sbs[(pn, pqt)]
                        )
                        if ph2 == NPAIR - 1:
                            fc_out(pn, pqt, attn_sbs[(pn, pqt)])
                    prev = (n, h2, qt, exps)
                    kT2_cur = kT2_next
                pn, ph2, pqt, pexps = prev
                flush_pair(pn, ph2, v_sbs[pn], pexps, attn_sbs[(pn, pqt)])
                fc_out(pn, pqt, attn_sbs[(pn, pqt)])

    nc.compile()
    return nc


def shard_inputs(values, keys, query, Wv, Wk, Wq, Wout, bout):
    import ml_dtypes

    bf16 = ml_dtypes.bfloat16
    f = np.float32
    values = np.asarray(values, dtype=f)
    keys = np.asarray(keys, dtype=f)
    query = np.asarray(query, dtype=f)
    Wv, Wk, Wq, Wout, bout = (np.asarray(x, dtype=f) for x in (Wv, Wk, Wq, Wout, bout))

    kT_full = np.ascontiguousarray(keys.transpose(0, 2, 1)).astype(bf16)
    qT_full = np.ascontiguousarray(query.transpose(0, 2, 1)).astype(bf16)

    # [V | 1] token-partition-major: vpack[n, p, c, h, :] =
    #   [values[n, c*128+p, h*64:(h+1)*64], 1]
    vpack = np.ones((N, 128, NCHUNK, H, D + 1), dtype=bf16)
    vr = values.reshape(N, NCHUNK, 128, H, D).transpose(0, 2, 1, 3, 4)
    vpack[:, :, :, :, 0:D] = vr.astype(bf16)

    Wc = (Wq.T @ Wk) / np.float32(np.sqrt(E))
    wpack = np.zeros((128, WCOLS), dtype=bf16)
    wpack[0:64, 0:64] = Wc.astype(bf16)
    wpack[64:128, 64:128] = Wc.astype(bf16)
    wvT = Wv.T.astype(bf16)
    wpack[0:64, WCOL_WV : WCOL_WV + 64] = wvT
    wpack[0:64, WCOL_WV + 192 : WCOL_WV + 256] = wvT
    wpack[:, WCOL_BIAS:WCOL_WOUT] = (
        bout.reshape(E // 128, 128).T.astype(bf16)
    )
    # wout block: [p, ec*E + o] = Wout.T[ec*128 + p, o]
    woutT = np.ascontiguousarray(Wout.T).astype(bf16)
    wpack[:, WCOL_WOUT:] = (
        woutT.reshape(E // 128, 128, E).transpose(1, 0, 2).reshape(128, -1)
    )

    shared = np.concatenate(
        [wpack.ravel(), kT_full.ravel(), vpack.ravel()])
    in_maps = []
    for c in range(NCORES):
        qc = np.ascontiguousarray(qT_full[:, :, c * LQ : (c + 1) * LQ])
        in_maps.append({
            "blob": np.concatenate([shared, qc.ravel()]),
        })
    return in_maps


def unshard(results):
    slabs = [np.asarray(r["outT"]).transpose(0, 2, 1) for r in results]
    return np.ascontiguousarray(np.concatenate(slabs, axis=1)).astype(np.float32)


def run_spmd(in_maps, **kwargs):
    from concourse.bass_utils import run_bass_kernel_spmd

    nc = build_nc()
    res = run_bass_kernel_spmd(nc, in_maps, core_ids=list(range(NCORES)), **kwargs)
    return nc, res


def kernel(**inputs):
    in_maps = shard_inputs(
        inputs["values"], inputs["keys"], inputs["query"],
        inputs["Wv"], inputs["Wk"], inputs["Wq"],
        inputs["Wout"], inputs["bout"],
    )
    _, res = run_spmd(in_maps)
    return unshard(res.results)


if __name__ == "__main__":
    rng = np.random.default_rng(0)
    ins = {
        "values": rng.standard_normal((N, L, E), dtype=np.float32),
        "keys": rng.standard_normal((N, L, E), dtype=np.float32),
        "query": rng.standard_normal((N, L, E), dtype=np.float32),
        "Wv": rng.standard_normal((D, D), dtype=np.float32) / 8,
        "Wk": rng.standard_normal((D, D), dtype=np.float32) / 8,
        "Wq": rng.standard_normal((D, D), dtype=np.float32) / 8,
        "Wout": rng.standard_normal((E, E), dtype=np.float32) / 32,
        "bout": rng.standard_normal((E,), dtype=np.float32) * 0.01,
    }
    out = kernel(**ins)
    print("out", out.shape, out.dtype, float(np.abs(out).max()))


# revision 16
# speedup vs baseline: 4.7720x; 1.0196x over previous
"""Trainium2 Bass kernel for nn_Attention (dense transformer attention).

Math (per batch n, head h):
  q' = q_h @ Wq.T ; k' = k_h @ Wk.T ; v' = v_h @ Wv.T
  S = (q' k'^T)/32 ; P = softmax_k(S) ; out_h = P v'
  final = concat_h(out_h) @ Wout.T + bout

Device-side reformulation (associativity, exact in real arithmetic):
  S   = Q @ Wc @ K^T      with Wc = (Wq.T @ Wk)/32   (K unprojected!)
  U^T = [V | 1]^T @ exp(S)^T   -> rows 0..63 = V^T exp(S)^T, row 64 = denoms
  out_h^T = (Wv @ U^T[0:64]) / denom    (Wv projection after attention)
  final^T = Wout @ attn^T + bout

Numerics: everything that feeds the PE is bf16 (|S| <= ~2.5 so exp is tame;
measured end-to-end absmax rel err ~4e-3, tolerance 2e-2). PSUM accumulation
stays f32, final output written f32.

Sharding: sequence-parallel over the 2048 queries -> NCORES cores x LQ
queries, processed in 256-query tiles. NCORES defaults to 4: the axon
dispatch path costs ~0.12 ms per core-execute per iteration, and with the
body ACT-bound at ~2.24 ms / NCORES, total per-iteration wall is minimized
at sqrt(2.24/0.12) ~= 4 cores (measured: 4 cores beat 8 at steady state).

Schedule: the (batch, head-pair, q-tile) slots are software-pipelined. In
slot p the PE computes S^T(p) (feeding ACT's exp, the bottleneck engine)
and then the U^T/Wv/normalize flush of slot p-1, whose exp outputs finished
during slot p-1. fc_out for a finished (batch, q-tile) runs right after the
next slot's S^T, so ACT keeps a full pair of exp work during fc_out's
matmuls. kT2 is loaded once per (batch, head-pair) and reused across
q-tiles.

Host-side packing makes every DMA big-descriptor and reduces the dispatch
cost to two buffers per core (blob in, outT out). blob layout (bf16):
  wpack (128, 8584)     all weights in one region:
        [:, 0:128]       blockdiag(Wc, Wc)
        [0:64, 128:384]  [Wv.T | 0] and [0 | Wv.T] (head0/head1 placement)
        [:, 384:392]     bias (bout partition-major)
        [:, 392:8584]    Wout^T as [128, ec, o]
  kT    (N, E, L)        keys^T, 4 KiB descriptors
  vpack (N, 128, NCHUNK, H, 65)  [V | 1] token-partition-major; 8 KiB
                         contiguous descriptor per partition per 4-chunk load
  qT    (N, E, LQ)       query^T slice, 512 B descriptors
"""

import sys

for p in ("/opt/trn_rl_repo",):
    if p not in sys.path:
        sys.path.insert(0, p)

import numpy as np

import os as _os

N = 2
L = 2048
E = 1024
H = 16
D = 64
NCORES = int(_os.environ.get("BASS_KERNEL_NCORES", "4"))
LQ = L // NCORES          # queries per core
LQB = 256                 # query-tile size (PSUM-sized)
QT = LQ // LQB            # query tiles per core
NPAIR = H // 2            # 8 head-pairs per batch
NCHUNK = L // 128         # 16 key chunks of 128 tokens
WCOL_WV = 128
WCOL_BIAS = 384
WCOL_WOUT = 392
WCOLS = 392 + E * (E // 128)

# single packed input blob (element offsets, bf16)
OFF_W = 0
OFF_K = OFF_W + 128 * WCOLS
OFF_V = OFF_K + N * E * L
OFF_Q = OFF_V + N * 128 * NCHUNK * H * (D + 1)
BLOB = OFF_Q + N * E * LQ

REPEAT = int(_os.environ.get("BASS_KERNEL_REPEAT", "1"))


def build_nc():
    import concourse.bass as bass
    import concourse.bacc as bacc
    import concourse.mybir as mybir
    import concourse.tile as tile

    f32 = mybir.dt.float32
    bf16 = mybir.dt.bfloat16
    EXP = mybir.ActivationFunctionType.Exp
    MUL = mybir.AluOpType.mult
    ADD = mybir.AluOpType.add

    nc = bacc.Bacc(None, target_bir_lowering=False, enable_partition_id=False)

    blob = nc.dram_tensor("blob", [BLOB], bf16, kind="ExternalInput")
    wpack = blob[OFF_W : OFF_W + 128 * WCOLS].rearrange(
        "(p c) -> p c", p=128, c=WCOLS)
    kT = blob[OFF_K : OFF_K + N * E * L].rearrange(
        "(n e l) -> n e l", n=N, e=E, l=L)
    vpack = blob[OFF_V : OFF_Q].rearrange(
        "(n p c h d) -> n p c h d", n=N, p=128, c=NCHUNK, h=H, d=D + 1)
    qT = blob[OFF_Q : OFF_Q + N * E * LQ].rearrange(
        "(n e l) -> n e l", n=N, e=E, l=LQ)
    outT = nc.dram_tensor("outT", [N, E, LQ], f32, kind="ExternalOutput")

    with tile.TileContext(nc) as tc:
        with (
            tc.tile_pool(name="const", bufs=1) as const,
            tc.tile_pool(name="vio", bufs=2) as vio,
            tc.tile_pool(name="io", bufs=2) as io,
            tc.tile_pool(name="work", bufs=3) as work,
            tc.tile_pool(name="expp", bufs=4) as expp,
            tc.tile_pool(name="attnp", bufs=QT + 1) as attnp,
            tc.tile_pool(name="psT", bufs=2, space="PSUM") as psT,
            tc.tile_pool(name="pu", bufs=2, space="PSUM") as pu,
            tc.tile_pool(name="psmall", bufs=2, space="PSUM") as psmall,
        ):
            # --- persistent constants: small weights (Wc, Wv, bias) first;
            # the 2 MiB Wout block is deferred until after the first pair's
            # loads so the S^T pipeline starts ~6 us earlier ---
            wpack_sb = const.tile([128, WCOLS], bf16)
            nc.sync.dma_start(wpack_sb[:, 0:WCOL_WOUT], wpack[:, 0:WCOL_WOUT])
            wqk2_sb = wpack_sb[:, 0:128]

            bias_sb = const.tile([128, E // 128], f32)
            nc.vector.tensor_copy(bias_sb, wpack_sb[:, WCOL_BIAS:WCOL_WOUT])
            ones_sb = const.tile([128, 128], bf16)
            nc.vector.memset(ones_sb, 1.0)

            import contextlib

            def load_k(n, h2):
                kT2 = io.tile([128, L], bf16, tag="kT2")
                nc.sync.dma_start(kT2, kT[n, 128 * h2 : 128 * (h2 + 1), :])
                return kT2

            def load_q(n, h2, qt):
                qT2 = io.tile([128, LQB], bf16, tag="qT2")
                nc.sync.dma_start(
                    qT2,
                    qT[n, 128 * h2 : 128 * (h2 + 1), LQB * qt : LQB * (qt + 1)],
                )
                return qT2

            def load_v(n):
                # 4 DMAs of 4 chunks each so early U^T chunks never wait on
                # the full 8 MiB load
                v_sb = vio.tile([128, NCHUNK, H, D + 1], bf16, tag="v",
                                name=f"v_sb_{n}")
                for g in range(4):
                    nc.sync.dma_start(
                        v_sb[:, 4 * g : 4 * (g + 1)], vpack[n, :, 4 * g : 4 * (g + 1)]
                    )
                return v_sb

            def score_phase(kT2, qT2):
                """Q'' projection, S^T matmuls and exp for one head pair."""
                pq = psmall.tile([128, LQB], f32, tag="small")
                nc.tensor.matmul(pq, wqk2_sb, qT2, start=True, stop=True)
                q2sb = work.tile([128, LQB], bf16, tag="q2sb")
                with nc.allow_low_precision("bf16 attention pipeline"):
                    nc.vector.tensor_copy(q2sb, pq)

                expS0 = expp.tile([128, NCHUNK, LQB], bf16, tag="expS")
                expS1 = expp.tile([128, NCHUNK, LQB], bf16, tag="expS")
                exps = (expS0, expS1)
                for rr in range(4):
                    sTs = []
                    for hh in range(2):
                        hs = slice(64 * hh, 64 * hh + 64)
                        sT = psT.tile([128, 4, LQB], f32, tag="sT")
                        sTs.append(sT)
                        for c in range(4):
                            ch = rr * 4 + c
                            nc.tensor.matmul(
                                sT[:, c, :],
                                kT2[hs, 128 * ch : 128 * (ch + 1)],
                                q2sb[hs, :],
                                start=True, stop=True,
                            )
                    for hh in range(2):
                        with nc.allow_low_precision("bf16 exp(S)"):
                            nc.scalar.activation(
                                exps[hh][:, rr * 4 : rr * 4 + 4, :],
                                sTs[hh][:, :, :], EXP,
                            )
                return exps

            def flush_pair(n, h2, v_sb, exps, attn_sb):
                """U^T accumulate, Wv projection, softmax normalize into
                attn_sb for a pair whose exp outputs are ready."""
                r2_sb = work.tile([65, 2, LQB], bf16, tag="r2")
                u_sbs = []
                for hh in range(2):
                    uT = pu.tile([65, LQB], f32, tag="uT")
                    for ch in range(NCHUNK):
                        nc.tensor.matmul(
                            uT,
                            v_sb[:, ch, 2 * h2 + hh, :],
                            exps[hh][:, ch, :],
                            start=(ch == 0), stop=(ch == NCHUNK - 1),
                        )
                    u_sb = work.tile([65, LQB], bf16, tag="u_sb")
                    u_sbs.append(u_sb)
                    with nc.allow_low_precision("bf16 attention pipeline"):
                        nc.vector.tensor_copy(u_sb, uT)
                        nc.vector.reciprocal(r2_sb[64:65, hh, :], u_sb[64:65, :])
                # Wv projection, head hh placed at partitions 64*hh..64*hh+63
                # via the zero-padded [Wv.T|0]/[0|Wv.T] stationary operands
                up = pu.tile([128, LQB], f32, tag="uT")
                for hh in range(2):
                    nc.tensor.matmul(
                        up,
                        wpack_sb[0:64, WCOL_WV + 128 * hh : WCOL_WV + 128 * (hh + 1)],
                        u_sbs[hh][0:64, :],
                        start=(hh == 0), stop=(hh == 1),
                    )
                # broadcast 1/denom across partitions via PE outer product
                pb = psmall.tile([128, 2, LQB], f32, tag="small")
                nc.tensor.matmul(
                    pb, ones_sb[64:65, :], r2_sb[64:65, :, :],
                    start=True, stop=True,
                )
                b_sb = work.tile([128, 2, LQB], bf16, tag="b_sb")
                with nc.allow_low_precision("bf16 attention pipeline"):
                    nc.vector.tensor_copy(b_sb, pb)
                    nc.vector.tensor_tensor(
                        attn_sb[0:64, h2, :], up[0:64, :], b_sb[0:64, 0, :], MUL,
                    )
                    nc.vector.tensor_tensor(
                        attn_sb[64:128, h2, :], up[64:128, :], b_sb[64:128, 1, :],
                        MUL,
                    )

            def fc_out(n, qt, attn_sb):
                for oc in range(E // 128):
                    po = psmall.tile([128, LQB], f32, tag="small")
                    for ec in range(E // 128):
                        nc.tensor.matmul(
                            po,
                            wpack_sb[:, WCOL_WOUT + E * ec + 128 * oc
                                     : WCOL_WOUT + E * ec + 128 * (oc + 1)],
                            attn_sb[:, ec, :],
                            start=(ec == 0), stop=(ec == E // 128 - 1),
                        )
                    o_sb = work.tile([128, LQB], f32, tag="o_sb")
                    nc.vector.tensor_tensor(
                        o_sb, po,
                        bias_sb[:, oc : oc + 1].to_broadcast((128, LQB)),
                        ADD,
                    )
                    nc.sync.dma_start(
                        outT[n, 128 * oc : 128 * (oc + 1),
                             LQB * qt : LQB * (qt + 1)], o_sb,
                    )

            rep_ctx = (
                tc.For_i(0, REPEAT, 1) if REPEAT > 1 else contextlib.nullcontext()
            )
            with rep_ctx:
                # slot order: batch -> head-pair -> query-tile, so kT2 is
                # loaded once per (n, h2) and reused across query tiles
                slots = [
                    (n, h2, qt)
                    for n in range(N) for h2 in range(NPAIR) for qt in range(QT)
                ]
                v_sbs = {}
                attn_sbs = {}
                # first pair's (small) loads go ahead of the 8 MiB v load so
                # the S^T pipeline starts immediately
                kT2_cur = load_k(*slots[0][:2])
                kT2_next = None
                loaded_q = load_q(*slots[0])
                nc.sync.dma_start(
                    wpack_sb[:, WCOL_WOUT:], wpack[:, WCOL_WOUT:]
                )
                v_sbs[0] = load_v(0)
                prev = None
                for idx, (n, h2, qt) in enumerate(slots):
                    if h2 == 0:
                        attn_sbs[(n, qt)] = attnp.tile(
                            [128, NPAIR, LQB], bf16, tag="attn",
                            name=f"attn_sb_{n}_{qt}",
                        )
                    qT2 = loaded_q
                    kT2_next = kT2_cur
                    if idx + 1 < len(slots):
                        nxt = slots[idx + 1]
                        if nxt[:2] != (n, h2):
                            kT2_next = load_k(*nxt[:2])
                        loaded_q = load_q(*nxt)
                    if n == 0 and h2 == NPAIR - 2 and qt == 0:
                        v_sbs[1] = load_v(1)
                    exps = score_phase(kT2_cur, qT2)
                    if prev is not None:
                        pn, ph2, pqt, pexps = prev
                        flush_pair(
                            pn, ph2, v_sbs[pn], pexps, attn_sbs[(pn, pqt)]
                        )
                        if ph2 == NPAIR - 1:
                            fc_out(pn, pqt, attn_sbs[(pn, pqt)])
                    prev = (n, h2, qt, exps)
                    kT2_cur = kT2_next
                pn, ph2, pqt, pexps = prev
                flush_pair(pn, ph2, v_sbs[pn], pexps, attn_sbs[(pn, pqt)])
                fc_out(pn, pqt, attn_sbs[(pn, pqt)])

    nc.compile()
    return nc


def shard_inputs(values, keys, query, Wv, Wk, Wq, Wout, bout):
    import ml_dtypes

    bf16 = ml_dtypes.bfloat16
    f = np.float32
    values = np.asarray(values, dtype=f)
    keys = np.asarray(keys, dtype=f)
    query = np.asarray(query, dtype=f)
    Wv, Wk, Wq, Wout, bout = (np.asarray(x, dtype=f) for x in (Wv, Wk, Wq, Wout, bout))

    kT_full = np.ascontiguousarray(keys.transpose(0, 2, 1)).astype(bf16)
    qT_full = np.ascontiguousarray(query.transpose(0, 2, 1)).astype(bf16)

    # [V | 1] token-partition-major: vpack[n, p, c, h, :] =
    #   [values[n, c*128+p, h*64:(h+1)*64], 1]
    vpack = np.ones((N, 128, NCHUNK, H, D + 1), dtype=bf16)
    vr = values.reshape(N, NCHUNK, 128, H, D).transpose(0, 2, 1, 3, 4)
    vpack[:, :, :, :, 0:D] = vr.astype(bf16)

    Wc = (Wq.T @ Wk) / np.float32(np.sqrt(E))
    wpack = np.zeros((128, WCOLS), dtype=bf16)
    wpack[0:64, 0:64] = Wc.astype(bf16)
    wpack[64:128, 64:128] = Wc.astype(bf16)
    wvT = Wv.T.astype(bf16)
    wpack[0:64, WCOL_WV : WCOL_WV + 64] = wvT
    wpack[0:64, WCOL_WV + 192 : WCOL_WV + 256] = wvT
    wpack[:, WCOL_BIAS:WCOL_WOUT] = (
        bout.reshape(E // 128, 128).T.astype(bf16)
    )
    # wout block: [p, ec*E + o] = Wout.T[ec*128 + p, o]
    woutT = np.ascontiguousarray(Wout.T).astype(bf16)
    wpack[:, WCOL_WOUT:] = (
        woutT.reshape(E // 128, 128, E).transpose(1, 0, 2).reshape(128, -1)
    )

    shared = np.concatenate(
        [wpack.ravel(), kT_full.ravel(), vpack.ravel()])
    in_maps = []
    for c in range(NCORES):
        qc = np.ascontiguousarray(qT_full[:, :, c * LQ : (c + 1) * LQ])
        in_maps.append({
            "blob": np.concatenate([shared, qc.ravel()]),
        })
    return in_maps


def unshard(results):
    slabs = [np.asarray(r["outT"]).transpose(0, 2, 1) for r in results]
    return np.ascontiguousarray(np.concatenate(slabs, axis=1)).astype(np.float32)


def run_spmd(in_maps, **kwargs):
    from concourse.bass_utils import run_bass_kernel_spmd

    nc = build_nc()
    res = run_bass_kernel_spmd(nc, in_maps, core_ids=list(range(NCORES)), **kwargs)
    return nc, res


def kernel(**inputs):
    in_maps = shard_inputs(
        inputs["values"], inputs["keys"], inputs["query"],
        inputs["Wv"], inputs["Wk"], inputs["Wq"],
        inputs["Wout"], inputs["bout"],
    )
    _, res = run_spmd(in_maps)
    return unshard(res.results)


if __name__ == "__main__":
    rng = np.random.default_rng(0)
    ins = {
        "values": rng.standard_normal((N, L, E), dtype=np.float32),
        "keys": rng.standard_normal((N, L, E), dtype=np.float32),
        "query": rng.standard_normal((N, L, E), dtype=np.float32),
        "Wv": rng.standard_normal((D, D), dtype=np.float32) / 8,
        "Wk": rng.standard_normal((D, D), dtype=np.float32) / 8,
        "Wq": rng.standard_normal((D, D), dtype=np.float32) / 8,
        "Wout": rng.standard_normal((E, E), dtype=np.float32) / 32,
        "bout": rng.standard_normal((E,), dtype=np.float32) * 0.01,
    }
    out = kernel(**ins)
    print("out", out.shape, out.dtype, float(np.abs(out).max()))
